# revision 12
# baseline (speedup 1.0000x reference)
"""Trainium2 Bass kernel for nn_CVRP_Encoder (AFT-style CVRP encoder).

Data-parallel over batch B=32 across 8 NeuronCores (4 items/core). Per item
everything lives in a transposed [D=128 (partitions), S=1000 (free)] layout so
instance-norm reduces along the free axis.

Main design points:
 - es = exp(-c*dist) precomputed on host, shipped fp8-e4m3 in a DoubleRow
   pair-grouped layout; ek = exp(k-4), ekv = ek*v in e5m2 (the global exp
   shift cancels in num/den). Attention contraction runs double-pumped fp8:
   2 contraction rows/cycle.
 - ACT function mix stays inside one table set (exp_and_others: tanh, exp,
   square, relu, identity) so there are no ACT_TABLE_LOAD switches.
 - sigmoid(q)*wgt is one DVE affine_mul_reduce: (0.5*tanh(q/2)+0.5)*wgt,
   which also emits sum(aft) for the norm-1 mean (analytic residual mean).
 - k and v share one stationary pass: rhs = [Wk.T | Wv.T] (256 wide), halving
   k/v LDWEIGHTS traffic.
 - norm scalar math (magic-rsqrt + 2 Newton) runs on GPSIMD so DVE/ACT stay
   free; norm-apply also on GPSIMD -> A/C never cross engines.
 - Per-layer weight blob = one DMA per layer.
"""
import sys

sys.path.insert(0, "/opt/trn_rl_repo")

import numpy as np

import concourse.bass as bass
import concourse.tile as tile
from concourse import bacc, mybir
from concourse.bass_utils import run_bass_kernel_spmd

F32 = mybir.dt.float32
F16 = mybir.dt.float16
BF16 = mybir.dt.bfloat16
F8E4 = mybir.dt.float8e4
F8E5 = mybir.dt.float8e5
I32 = mybir.dt.int32
AF = mybir.ActivationFunctionType
ALU = mybir.AluOpType
DR = mybir.MatmulPerfMode.DoubleRow

B, N, D, F, L = 32, 999, 128, 512, 6
S = N + 1
P = 128
NCORES = 8
IPC = B // NCORES
TC = 8             # t-chunks of 125 (partition dim of attention contraction)
TCS = S // TC      # 125
PC = 4             # DoubleRow pair-chunks (2 t-chunks each)
SC = 2
SCS = S // SC      # 500
FC = F // P        # 4
EPS = 1e-5
KSHIFT = 4.0       # ek = exp(k - KSHIFT); cancels in num/den
GRP = 2            # items per norm-batching group
RSQRT_MAGIC = 0x5F3759DF + 1
WCOLS = 3 * D + 2 * F          # per-layer weight blob columns (1408)


def _bcast_dram(handle, n_part, idx, count):
    ap = handle[:]
    return bass.AP(tensor=ap.tensor, offset=idx, ap=[[0, n_part], [1, count]])


def _nv(t):
    """[P, 1024] tile/psum -> [P, 2, 500] strided view (skip 512-pad)."""
    return t[:].rearrange("p (n s) -> p n s", n=2)[:, :, 0:SCS]


def _v2(t):
    """[P, S] tile -> [P, 2, 500] view."""
    return t[:].rearrange("p (n s) -> p n s", n=2)


def build_cvrp(cs):
    """cs: per-layer scale constants c_l = log_scale * alpha[l]."""
    shared_es = all(abs(c - cs[0]) < 1e-30 for c in cs)
    n_es = 1 if shared_es else L

    nc = bacc.Bacc("TRN2", target_bir_lowering=False, debug=False,
                   num_devices=NCORES)

    g = {}
    g["es8"] = nc.declare_dram_parameter("es8", [n_es, IPC, TCS, TC, S], F8E4, isOutput=False)
    g["node_t"] = nc.declare_dram_parameter("node_t", [IPC, 3, N], F16, isOutput=False)
    g["depot"] = nc.declare_dram_parameter("depot", [IPC, 2], F32, isOutput=False)
    g["flagf"] = nc.declare_dram_parameter("flagf", [IPC], F32, isOutput=False)
    g["wblob"] = nc.declare_dram_parameter("wblob", [L, D, WCOLS], F16, isOutput=False)
    g["wnt"] = nc.declare_dram_parameter("wnt", [3, D], F16, isOutput=False)
    g["wdt"] = nc.declare_dram_parameter("wdt", [2, D], F32, isOutput=False)
    g["wint"] = nc.declare_dram_parameter("wint", [D, D], F32, isOutput=False)
    g["woutt"] = nc.declare_dram_parameter("woutt", [D, D], F32, isOutput=False)
    # sblob: biases4(4) | bw1 (L*FC=24) | g1(6) | b1(6) | g2(6) | b2(6) = 52
    g["sblob"] = nc.declare_dram_parameter("sblob", [D, 52], F32, isOutput=False)
    g["out32"] = nc.declare_dram_parameter("out32", [IPC, D, S], F32, isOutput=True)

    with tile.TileContext(nc) as tc_ctx:
        _body(nc, tc_ctx, g, cs, shared_es)
    nc.compile()
    return nc


def _norm_smalls(nc, np_, sums, sumsq, g_col, b_col, tag, mean_bias=None,
                 mean_bias_cols=None):
    """Instance-norm scalar math on [D, GRP] tiles, on GPSIMD.
    mean = sums/S (+bias); var = sumsq/S + eps - mean^2; rstd via magic
    rsqrt + 2 Newton iters. Returns (A, C): out = A*y + C."""
    e = nc.gpsimd
    sm = np_.tile([D, 8, GRP], F32, tag=f"nsm_{tag}")
    mean, msq, var = sm[:, 0], sm[:, 1], sm[:, 2]
    if mean_bias is not None:
        e.tensor_scalar(mean, sums, 1.0 / S, mean_bias, ALU.mult, ALU.add)
    else:
        e.tensor_scalar(mean, sums, 1.0 / S, None, ALU.mult)
    if mean_bias_cols is not None:
        e.tensor_tensor(mean, mean, mean_bias_cols, ALU.add)
    e.tensor_tensor(msq, mean, mean, ALU.mult)
    e.tensor_scalar(var, sumsq, 1.0 / S, EPS, ALU.mult, ALU.add)
    e.tensor_tensor(var, var, msq, ALU.subtract)
    ry = sm[:, 3]
    ibits = ry.bitcast(I32)
    # int bit-trick ops are not supported on Pool; run them on DVE
    nc.vector.tensor_scalar(ibits, var.bitcast(I32), 1, -1,
                            ALU.logical_shift_right, ALU.bitwise_xor)
    nc.vector.tensor_scalar(ibits, ibits, RSQRT_MAGIC, None, ALU.add)
    t1, t2 = sm[:, 4], sm[:, 5]
    for _ in range(2):
        e.tensor_tensor(t1, ry, ry, ALU.mult)
        e.tensor_tensor(t2, t1, var, ALU.mult)
        e.tensor_scalar(t2, t2, -0.5, 1.5, ALU.mult, ALU.add)
        e.tensor_tensor(ry, ry, t2, ALU.mult)
    A, C = sm[:, 6], sm[:, 7]
    e.tensor_scalar(A, ry, g_col, None, ALU.mult)
    e.tensor_tensor(C, mean, A, ALU.mult)
    e.tensor_scalar(C, C, b_col, -1.0, ALU.subtract, ALU.mult)
    return A, C


def _body(nc, tc, g, cs, shared_es):
    from contextlib import ExitStack

    ctx = ExitStack()
    singles = ctx.enter_context(tc.tile_pool(name="singles", bufs=1))
    xpool = ctx.enter_context(tc.tile_pool(name="xpool", bufs=1))
    tp = ctx.enter_context(tc.tile_pool(name="tp", bufs=2))
    scr = ctx.enter_context(tc.tile_pool(name="scr", bufs=2))
    np_ = ctx.enter_context(tc.tile_pool(name="npool", bufs=2))
    pp = ctx.enter_context(tc.tile_pool(name="pp", bufs=1))
    ps = ctx.enter_context(tc.tile_pool(name="ps", bufs=4, space="PSUM"))

    # ---- embedding weights + per-item inputs first (gate the pipeline) ----
    t_wnt = singles.tile([3, D], F16, tag="wnt")
    nc.sync.dma_start(t_wnt[:], g["wnt"][:])
    t_wdt = singles.tile([2, D], F32, tag="wdt")
    nc.sync.dma_start(t_wdt[:], g["wdt"][:])
    t_wint = singles.tile([D, D], F32, tag="wint")
    nc.sync.dma_start(t_wint[:], g["wint"][:])
    t_woutt = singles.tile([D, D], F32, tag="woutt")
    nc.sync.dma_start(t_woutt[:], g["woutt"][:])
    t_sb = singles.tile([D, 52], F32, tag="sblob")
    nc.sync.dma_start(t_sb[:], g["sblob"][:])
    t_b4 = t_sb[:, 0:4]
    t_bw1 = t_sb[:, 4:28].rearrange("p (l f) -> p l f", l=L)
    t_g1, t_b1 = t_sb[:, 28:34], t_sb[:, 34:40]
    t_g2, t_b2 = t_sb[:, 40:46], t_sb[:, 46:52]
    t_ff = singles.tile([P, IPC], F32, tag="ffl")
    nc.sync.dma_start(t_ff[:], _bcast_dram(g["flagf"], P, 0, IPC))
    t_const = singles.tile([P, 1], F32, tag="consts")
    nc.gpsimd.memset(t_const[:, 0:1], -KSHIFT)
    KB = t_const[:, 0:1]

    # ---- layer-0 weights, then es8, then remaining layers ----
    t_wb = []

    def load_layer_weights(l):
        w = singles.tile([D, WCOLS], F16, tag=f"wb_{l}", name=f"wb_{l}")
        nc.sync.dma_start(w[:], g["wblob"][l])
        t_wb.append(w)

    load_layer_weights(0)
    es_tiles = {}
    if shared_es:
        for i in range(IPC):
            es = singles.tile([TCS, TC, S], F8E4, tag=f"es{i}", name=f"es{i}")
            nc.sync.dma_start(es[:, 0:4], g["es8"][0, i, :, 0:4])
            nc.sync.dma_start(es[:, 4:8], g["es8"][0, i, :, 4:8])
            es_tiles[i] = es
    for l in range(1, L):
        load_layer_weights(l)

    def wq(l):
        return t_wb[l][:, 0:D]

    def wkv(l):
        return t_wb[l][:, D : 3 * D]

    def w1(l, fc):
        return t_wb[l][:, 3 * D + fc * P : 3 * D + (fc + 1) * P]

    def w2(l, fc):
        return t_wb[l][:, 3 * D + F + fc * P : 3 * D + F + (fc + 1) * P]

    BD, BN_, BIN, BOUT = (t_b4[:, i : i + 1] for i in range(4))
    emb_mean = singles.tile([D, IPC], F32, tag="embm")

    # ---- embedding ----
    x16s = []
    for i in range(IPC):
        x32 = xpool.tile([D, S], F32, tag=f"x32_{i}", name=f"x32_{i}")
        t_node = scr.tile([P, 1024], F16, tag="node16")
        nc.sync.dma_start(t_node[:3, 0:N], g["node_t"][i])
        t_dep = tp.tile([2, 1], F32, tag="dep")
        nc.sync.dma_start(t_dep[:], g["depot"][i, :, None])
        pe = ps.tile([P, 1024], F32, tag="ps")
        nc.tensor.matmul(pe[:, 0:500], t_wnt[:], t_node[:3, 0:500], start=True, stop=True)
        nc.tensor.matmul(pe[:, 512:1011], t_wnt[:], t_node[:3, 500:999], start=True, stop=True)
        nc.scalar.activation(x32[:, 1:501], pe[:, 0:500], AF.Identity, bias=BN_, scale=1.0)
        nc.scalar.activation(x32[:, 501:1000], pe[:, 512:1011], AF.Identity, bias=BN_, scale=1.0)
        pd = ps.tile([P, 1024], F32, tag="ps")
        nc.tensor.matmul(pd[:, 0:1], t_wdt[:], t_dep[:], start=True, stop=True)
        nc.scalar.activation(x32[:, 0:1], pd[:, 0:1], AF.Identity, bias=BD, scale=1.0)
        pw = ps.tile([P, 1024], F32, tag="ps")
        nc.tensor.matmul(pw[:, 0:1], t_wint[:], x32[:, 1:2], start=True, stop=True)
        nc.scalar.activation(x32[:, 1:2], pw[:, 0:1], AF.Identity, bias=BIN, scale=1.0)
        # flag row fix: u = f*x0 + (1-f)*x999 ; w = Wout@u + bout ;
        # x0 += f*(w-u) ; x999 += (1-f)*(w-u)
        fcol = t_ff[:, i : i + 1]
        sm = np_.tile([D, 8], F32, tag="flagtmp")
        d1, u, t2, w_sb, d0 = (sm[:, j : j + 1] for j in range(5))
        nc.vector.tensor_tensor(d1, x32[:, 0:1], x32[:, 999:1000], ALU.subtract)
        nc.vector.tensor_scalar(d1, d1, fcol, None, ALU.mult)
        nc.vector.tensor_tensor(u, x32[:, 999:1000], d1, ALU.add)
        pf = ps.tile([P, 1024], F32, tag="ps")
        nc.tensor.matmul(pf[:, 0:1], t_woutt[:], u, start=True, stop=True)
        nc.scalar.activation(w_sb, pf[:, 0:1], AF.Identity, bias=BOUT, scale=1.0)
        nc.vector.tensor_tensor(t2, w_sb, u, ALU.subtract)
        nc.vector.tensor_scalar(d0, t2, fcol, None, ALU.mult)
        nc.vector.tensor_tensor(x32[:, 0:1], x32[:, 0:1], d0, ALU.add)
        nc.vector.tensor_tensor(x32[:, 999:1000], x32[:, 999:1000], t2, ALU.add)
        nc.vector.tensor_tensor(x32[:, 999:1000], x32[:, 999:1000], d0, ALU.subtract)
        x16 = xpool.tile([D, S], F16, tag=f"x16_{i}", name=f"x16_{i}")
        nc.vector.tensor_copy(x16[:], x32[:])
        nc.vector.tensor_reduce(emb_mean[:, i : i + 1], x32[:], axis=mybir.AxisListType.X, op=ALU.add)
        x16s.append(x16)
    nc.vector.tensor_scalar(emb_mean[:], emb_mean[:], 1.0 / S, None, ALU.mult)

    # ---- encoder layers ----
    for l in range(L):
        wgts, afts, ys, h16s, y2s = {}, {}, {}, {}, {}
        sts = {}
        tqs, eks, ekvs = {}, {}, {}

        def a1(i):
            """q + k|v matmuls, tanh, ek, ekv for item i."""
            g0 = (i // GRP) * GRP
            if i == g0:
                sts[g0] = (np_.tile([D, 2, GRP], F32, tag=f"st1_{g0}", name=f"st1_{g0}"),
                           np_.tile([D, 2, GRP], F32, tag=f"st2_{g0}", name=f"st2_{g0}"))
            x16 = x16s[i]
            pq = ps.tile([P, 1024], F32, tag="ps")
            nc.tensor.matmul(pq[:, 0:500], wq(l), x16[:, 0:500], start=True, stop=True)
            nc.tensor.matmul(pq[:, 512:1012], wq(l), x16[:, 500:1000], start=True, stop=True)
            pkv_a = ps.tile([P, 1024], F32, tag="ps")
            pkv_b = ps.tile([P, 1024], F32, tag="ps")
            for c in range(TC):
                pkv = pkv_a if c < 4 else pkv_b
                cc = c % 4
                nc.tensor.matmul(pkv[:TCS, cc * 256 : (cc + 1) * 256],
                                 x16[:, c * TCS : (c + 1) * TCS], wkv(l),
                                 start=True, stop=True)
            tq = tp.tile([P, S], F16, tag="tq")
            nc.scalar.activation(_v2(tq), _nv(pq), AF.Tanh, bias=0.0, scale=0.5)
            tqs[i] = tq
            ek = tp.tile([P, TC * P], F8E5, tag="ek")
            ekv = tp.tile([P, TC * P], F8E5, tag="ekv")
            for h, pkv in ((0, pkv_a), (1, pkv_b)):
                pv4 = pkv[:TCS].rearrange("p (c two d) -> p c two d", two=2, d=P)
                eko = ek[:TCS, h * 512 : (h + 1) * 512].rearrange("p (c d) -> p c d", d=P)
                ekvo = ekv[:TCS, h * 512 : (h + 1) * 512].rearrange("p (c d) -> p c d", d=P)
                nc.scalar.activation(eko, pv4[:, :, 0], AF.Exp, bias=KB[:TCS], scale=1.0)
                nc.vector.tensor_tensor(ekvo, eko, pv4[:, :, 1], ALU.mult)
            eks[i], ekvs[i] = ek, ekv

        def a2(i):
            """den/num DoubleRow matmuls, reciprocal, wgt, aft for item i."""
            g0 = (i // GRP) * GRP
            ek, ekv = eks[i], ekvs[i]
            if shared_es:
                es = es_tiles[i]
            else:
                es = tp.tile([TCS, TC, S], F8E4, tag="es_dyn")
                nc.sync.dma_start(es[:], g["es8"][l, i])
            pden = ps.tile([P, 1024], F32, tag="ps")
            for sc in range(SC):
                off = sc * 512
                ssl = slice(sc * SCS, (sc + 1) * SCS)
                for c in range(PC):
                    nc.tensor.matmul(pden[:, off : off + SCS],
                                     ek[:TCS, c * 2 * P : (c + 1) * 2 * P].rearrange("p (j d) -> p j d", j=2),
                                     es[:, 2 * c : 2 * c + 2, ssl],
                                     start=(c == 0), stop=(c == PC - 1), perf_mode=DR)
            rden = scr.tile([P, 1024], F32, tag="rden")
            nc.vector.reciprocal_approx_fast(out=_nv(rden), in_=_nv(pden))
            pnum = ps.tile([P, 1024], F32, tag="ps")
            for sc in range(SC):
                off = sc * 512
                ssl = slice(sc * SCS, (sc + 1) * SCS)
                for c in range(PC):
                    nc.tensor.matmul(pnum[:, off : off + SCS],
                                     ekv[:TCS, c * 2 * P : (c + 1) * 2 * P].rearrange("p (j d) -> p j d", j=2),
                                     es[:, 2 * c : 2 * c + 2, ssl],
                                     start=(c == 0), stop=(c == PC - 1), perf_mode=DR)
            wgt = pp.tile([P, S], F16, tag=f"wgt_{i}", name=f"wgt_{i}")
            nc.vector.tensor_tensor(_v2(wgt), _nv(pnum), _nv(rden), ALU.mult)
            wgts[i] = wgt
            st1 = sts[g0][0]
            aft = pp.tile([P, S], F16, tag=f"aft_{i}", name=f"aft_{i}")
            nc.vector.affine_mul_reduce(out=aft[:], accum_out=st1[:, 0, (i - g0) : (i - g0) + 1],
                                        in0=tqs[i][:], in1=wgt[:], scale=0.5, bias=0.5)
            afts[i] = aft

        def phase_b(g0):
            """norm-1 for group g0: y, sumsq, smalls, h16."""
            st1 = sts[g0][0]
            for i in range(g0, g0 + GRP):
                j = i - g0
                y = xpool.tile([P, S], F32, tag=f"y_{i}", name=f"y_{i}")
                nc.vector.tensor_tensor(y[:], x16s[i][:], afts[i][:], ALU.add)
                ys[i] = y
                sq = scr.tile([P, 1024], F32, tag="sqscr")
                nc.scalar.activation(sq[:, 0:S], y[:], AF.Square, accum_out=st1[:, 1, j : j + 1])
            mb = t_b2[:, l - 1 : l] if l > 0 else None
            mbc = emb_mean[:, g0 : g0 + GRP] if l == 0 else None
            A1, C1 = _norm_smalls(nc, np_, st1[:, 0], st1[:, 1],
                                  t_g1[:, l : l + 1], t_b1[:, l : l + 1], f"n1_{g0}",
                                  mean_bias=mb, mean_bias_cols=mbc)
            for i in range(g0, g0 + GRP):
                j = i - g0
                h16 = pp.tile([P, S], F16, tag=f"h16_{i}", name=f"h16_{i}")
                nc.gpsimd.tensor_scalar(h16[:], ys[i][:], A1[:, j : j + 1], C1[:, j : j + 1],
                                        ALU.mult, ALU.add)
                h16s[i] = h16

        def phase_c(i):
            """FFN + y2 + sumsq for item i (fc-pipelined emission)."""
            g0 = (i // GRP) * GRP
            j = i - g0
            st2 = sts[g0][1]
            h16 = h16s[i]
            pf2 = ps.tile([P, 1024], F32, tag="ps")
            pf1s = [ps.tile([P, 1024], F32, tag="ps", name=f"pf1_{fc}") for fc in range(FC)]
            r16s = [None] * FC

            def emit_w1(fc):
                pf1 = pf1s[fc]
                nc.tensor.matmul(pf1[:, 0:500], w1(l, fc), h16[:, 0:500], start=True, stop=True)
                nc.tensor.matmul(pf1[:, 512:1012], w1(l, fc), h16[:, 500:1000], start=True, stop=True)
                r16 = tp.tile([P, S], F16, tag="r16")
                bcol = t_bw1[:, l, fc : fc + 1]
                if fc % 2 == 0:
                    nc.scalar.activation(_v2(r16), _nv(pf1), AF.Relu, bias=bcol, scale=1.0)
                else:
                    nc.vector.tensor_scalar(_v2(r16), _nv(pf1), bcol, 0.0, ALU.add, ALU.max)
                r16s[fc] = r16

            def emit_w2(fc):
                for sc in range(SC):
                    nc.tensor.matmul(pf2[:, sc * 512 : sc * 512 + SCS],
                                     w2(l, fc), r16s[fc][:, sc * SCS : (sc + 1) * SCS],
                                     start=(fc == 0), stop=(fc == FC - 1))

            emit_w1(0)
            emit_w1(1)
            emit_w2(0)
            emit_w1(2)
            emit_w2(1)
            emit_w1(3)
            emit_w2(2)
            emit_w2(3)
            # y2 = h + ff  (bW2 cancels in next norm)
            y2 = xpool.tile([P, S], F32, tag=f"y2_{i}", name=f"y2_{i}")
            nc.vector.scalar_tensor_tensor(_v2(y2), _v2(h16), 0.0, _nv(pf2),
                                           ALU.add, ALU.add,
                                           accum_out=st2[:, 0, j : j + 1])
            y2s[i] = y2
            sq = scr.tile([P, 1024], F32, tag="sqscr")
            nc.scalar.activation(sq[:, 0:S], y2[:], AF.Square, accum_out=st2[:, 1, j : j + 1])

        def phase_d(g0):
            """norm-2 for group g0: smalls + next-layer x16 (or output)."""
            st2 = sts[g0][1]
            A2, C2 = _norm_smalls(nc, np_, st2[:, 0], st2[:, 1],
                                  t_g2[:, l : l + 1], t_b2[:, l : l + 1], f"n2_{g0}")
            for i in range(g0, g0 + GRP):
                j = i - g0
                if l < L - 1:
                    nx16 = xpool.tile([D, S], F16, tag=f"x16_{i}{'b' if l % 2 == 0 else ''}",
                                      name=f"nx16_{i}")
                    nc.gpsimd.tensor_scalar(nx16[:], y2s[i][:], A2[:, j : j + 1], C2[:, j : j + 1],
                                            ALU.mult, ALU.add)
                    x16s[i] = nx16
                else:
                    xout = xpool.tile([D, S], F32, tag=f"x32_{i}", name=f"xout_{i}")
                    nc.gpsimd.tensor_scalar(xout[:], y2s[i][:], A2[:, j : j + 1], C2[:, j : j + 1],
                                            ALU.mult, ALU.add)
                    nc.sync.dma_start(g["out32"][i], xout[:])

        # software-pipelined emission: the PE queue always holds work that
        # does not depend on the in-flight DVE/ACT/POOL chains.
        a1(0)
        a1(1)
        a2(0)
        a1(2)
        a2(1)
        phase_b(0)
        a1(3)
        a2(2)
        a2(3)
        phase_c(0)
        phase_b(2)
        phase_c(1)
        phase_d(0)
        phase_c(2)
        phase_c(3)
        phase_d(2)

    ctx.close()


# ------------------------------------------------------------------
# host wrapper
# ------------------------------------------------------------------
_cache = {}


def _get_nc(cs_key):
    if cs_key not in _cache:
        _cache[cs_key] = build_cvrp(list(cs_key))
    return _cache[cs_key]


def prep_inputs(depot_xy, node_xy_demand, dist, log_scale, flag,
                Wd, bd, Wn, bn, Win, bin_, Wout, bout,
                Wq, Wk, Wv, alpha, g1, b1, W1, bW1, W2, bW2, g2, b2):
    import ml_dtypes

    flag = np.asarray(flag)
    cs = tuple(float(np.asarray(log_scale)[0]) * float(a) for a in np.asarray(alpha))
    shared_es = all(abs(c - cs[0]) < 1e-30 for c in cs)

    # es8[b, p, cc, s] = exp(-c_l * dist[b, s, cc*125+p]), fp8-e4m3,
    # cc = consecutive 125-row t-chunks (DoubleRow pairs are (2c, 2c+1)).
    dist_t = np.asarray(dist).transpose(0, 2, 1).reshape(B, TC, TCS, S)
    layers = [cs[0]] if shared_es else list(cs)
    es8 = np.empty((len(layers), B, TCS, TC, S), dtype=ml_dtypes.float8_e4m3)
    for li, c in enumerate(layers):
        es8[li] = np.exp(-c * dist_t).transpose(0, 2, 1, 3).astype(ml_dtypes.float8_e4m3)

    node_t = np.ascontiguousarray(np.asarray(node_xy_demand).transpose(0, 2, 1)).astype(np.float16)
    depot = np.asarray(depot_xy).reshape(B, 2).astype(np.float32)
    flagf = flag.astype(np.float32)

    f16 = lambda a: np.ascontiguousarray(np.asarray(a)).astype(np.float16)
    f32 = lambda a: np.ascontiguousarray(np.asarray(a)).astype(np.float32)
    # per-layer weight blob: [L, D, 1408] = WqT | WkT | WvT | W1T | W2blob
    w2b = np.asarray(W2).transpose(0, 2, 1).reshape(L, FC, P, D).transpose(0, 2, 1, 3).reshape(L, P, FC * D)
    wblob = np.concatenate([
        np.asarray(Wq).transpose(0, 2, 1),
        np.asarray(Wk).transpose(0, 2, 1),
        np.asarray(Wv).transpose(0, 2, 1),
        np.asarray(W1).transpose(0, 2, 1),
        w2b,
    ], axis=2).astype(np.float16)
    sblob = np.concatenate([
        np.stack([np.asarray(bd), np.asarray(bn), np.asarray(bin_), np.asarray(bout)], axis=1),
        np.asarray(bW1).reshape(L, FC, P).transpose(2, 0, 1).reshape(P, L * FC),
        np.asarray(g1).T, np.asarray(b1).T, np.asarray(g2).T, np.asarray(b2).T,
    ], axis=1).astype(np.float32)
    shared = {
        "wblob": np.ascontiguousarray(wblob),
        "wnt": f16(np.asarray(Wn).T),
        "wdt": f32(np.asarray(Wd).T),
        "wint": f32(np.asarray(Win).T),
        "woutt": f32(np.asarray(Wout).T),
        "sblob": np.ascontiguousarray(sblob),
    }
    in_maps = []
    for c in range(NCORES):
        sl = slice(c * IPC, (c + 1) * IPC)
        m = dict(shared)
        m["es8"] = np.ascontiguousarray(es8[:, sl])
        m["node_t"] = node_t[sl]
        m["depot"] = depot[sl]
        m["flagf"] = flagf[sl]
        in_maps.append(m)
    return cs, in_maps


TRACE = False
LAST_RESULT = None


def kernel(**inputs):
    global LAST_RESULT
    cs, in_maps = prep_inputs(**inputs)
    nc = _get_nc(cs)
    res = run_bass_kernel_spmd(nc, in_maps, list(range(NCORES)), trace=TRACE)
    LAST_RESULT = res
    out = np.concatenate([r["out32"] for r in res.results], axis=0)  # [B, D, S]
    return np.ascontiguousarray(out.transpose(0, 2, 1)).astype(np.float32)


# revision 13
# speedup vs baseline: 1.1190x; 1.1190x over previous
"""Trainium2 Bass kernel for nn_CVRP_Encoder (AFT-style CVRP encoder).

Data-parallel over batch B=32 across 8 NeuronCores (4 items/core). Per item
everything lives in a transposed [D=128 (partitions), S=1000 (free)] layout so
instance-norm reduces along the free axis.

Main design points:
 - es = exp(-c*dist) precomputed on host, shipped fp8-e4m3 in a DoubleRow
   pair-grouped layout; ek = exp(k-4), ekv = ek*v in e5m2 (the global exp
   shift cancels in num/den). Attention contraction runs double-pumped fp8:
   2 contraction rows/cycle.
 - ACT function mix stays inside one table set (exp_and_others: tanh, exp,
   square, relu, identity) so there are no ACT_TABLE_LOAD switches.
 - sigmoid(q)*wgt is one DVE affine_mul_reduce: (0.5*tanh(q/2)+0.5)*wgt,
   which also emits sum(aft) for the norm-1 mean (analytic residual mean).
 - k and v share one stationary pass: rhs = [Wk.T | Wv.T] (256 wide), halving
   k/v LDWEIGHTS traffic.
 - norm scalar math (magic-rsqrt + 2 Newton) runs on GPSIMD so DVE/ACT stay
   free; norm-apply also on GPSIMD -> A/C never cross engines.
 - Per-layer weight blob = one DMA per layer.
"""
import sys

sys.path.insert(0, "/opt/trn_rl_repo")

import numpy as np

import concourse.bass as bass
import concourse.tile as tile
from concourse import bacc, mybir
from concourse.bass_utils import run_bass_kernel_spmd

F32 = mybir.dt.float32
F16 = mybir.dt.float16
BF16 = mybir.dt.bfloat16
F8E4 = mybir.dt.float8e4
F8E5 = mybir.dt.float8e5
I32 = mybir.dt.int32
AF = mybir.ActivationFunctionType
ALU = mybir.AluOpType
DR = mybir.MatmulPerfMode.DoubleRow

B, N, D, F, L = 32, 999, 128, 512, 6
S = N + 1
P = 128
NCORES = 8
IPC = B // NCORES
TC = 8             # t-chunks of 125 (partition dim of attention contraction)
TCS = S // TC      # 125
PC = 4             # DoubleRow pair-chunks (2 t-chunks each)
SC = 2
SCS = S // SC      # 500
FC = F // P        # 4
EPS = 1e-5
KSHIFT = 4.0       # ek = exp(k - KSHIFT); cancels in num/den
GRP = 2            # items per norm-batching group
RSQRT_MAGIC = 0x5F3759DF + 1
WCOLS = 3 * D + 2 * F          # per-layer weight blob columns (1408)


def _bcast_dram(handle, n_part, idx, count):
    ap = handle[:]
    return bass.AP(tensor=ap.tensor, offset=idx, ap=[[0, n_part], [1, count]])


def _nv(t):
    """[P, 1024] tile/psum -> [P, 2, 500] strided view (skip 512-pad)."""
    return t[:].rearrange("p (n s) -> p n s", n=2)[:, :, 0:SCS]


def _v2(t):
    """[P, S] tile -> [P, 2, 500] view."""
    return t[:].rearrange("p (n s) -> p n s", n=2)


def build_cvrp(cs):
    """cs: per-layer scale constants c_l = log_scale * alpha[l]."""
    shared_es = all(abs(c - cs[0]) < 1e-30 for c in cs)
    n_es = 1 if shared_es else L

    nc = bacc.Bacc("TRN2", target_bir_lowering=False, debug=False,
                   num_devices=NCORES)

    g = {}
    g["es8"] = nc.declare_dram_parameter("es8", [n_es, IPC, TCS, TC, S], F8E4, isOutput=False)
    g["node_t"] = nc.declare_dram_parameter("node_t", [IPC, 3, N], F16, isOutput=False)
    g["depot"] = nc.declare_dram_parameter("depot", [IPC, 2], F32, isOutput=False)
    g["flagf"] = nc.declare_dram_parameter("flagf", [IPC], F32, isOutput=False)
    g["wblob"] = nc.declare_dram_parameter("wblob", [L, D, WCOLS], F16, isOutput=False)
    g["wnt"] = nc.declare_dram_parameter("wnt", [3, D], F16, isOutput=False)
    g["wdt"] = nc.declare_dram_parameter("wdt", [2, D], F32, isOutput=False)
    g["wint"] = nc.declare_dram_parameter("wint", [D, D], F32, isOutput=False)
    g["woutt"] = nc.declare_dram_parameter("woutt", [D, D], F32, isOutput=False)
    # sblob: biases4(4) | bw1 (L*FC=24) | g1(6) | b1(6) | g2(6) | b2(6) = 52
    g["sblob"] = nc.declare_dram_parameter("sblob", [D, 52], F32, isOutput=False)
    g["out32"] = nc.declare_dram_parameter("out32", [IPC, D, S], F32, isOutput=True)

    with tile.TileContext(nc) as tc_ctx:
        _body(nc, tc_ctx, g, cs, shared_es)
    nc.compile()
    return nc


def _norm_smalls(nc, np_, sums, sumsq, g_col, b_col, tag, mean_bias=None,
                 mean_bias_cols=None):
    """Instance-norm scalar math on [D, GRP] tiles, on GPSIMD.
    mean = sums/S (+bias); var = sumsq/S + eps - mean^2; rstd via magic
    rsqrt + 2 Newton iters. Returns (A, C): out = A*y + C."""
    e = nc.gpsimd
    sm = np_.tile([D, 8, GRP], F32, tag=f"nsm_{tag}")
    mean, msq, var = sm[:, 0], sm[:, 1], sm[:, 2]
    if mean_bias is not None:
        e.tensor_scalar(mean, sums, 1.0 / S, mean_bias, ALU.mult, ALU.add)
    else:
        e.tensor_scalar(mean, sums, 1.0 / S, None, ALU.mult)
    if mean_bias_cols is not None:
        e.tensor_tensor(mean, mean, mean_bias_cols, ALU.add)
    e.tensor_tensor(msq, mean, mean, ALU.mult)
    e.tensor_scalar(var, sumsq, 1.0 / S, EPS, ALU.mult, ALU.add)
    e.tensor_tensor(var, var, msq, ALU.subtract)
    ry = sm[:, 3]
    ibits = ry.bitcast(I32)
    # int bit-trick ops are not supported on Pool; run them on DVE
    nc.vector.tensor_scalar(ibits, var.bitcast(I32), 1, -1,
                            ALU.logical_shift_right, ALU.bitwise_xor)
    nc.vector.tensor_scalar(ibits, ibits, RSQRT_MAGIC, None, ALU.add)
    t1, t2 = sm[:, 4], sm[:, 5]
    for _ in range(2):
        e.tensor_tensor(t1, ry, ry, ALU.mult)
        e.tensor_tensor(t2, t1, var, ALU.mult)
        e.tensor_scalar(t2, t2, -0.5, 1.5, ALU.mult, ALU.add)
        e.tensor_tensor(ry, ry, t2, ALU.mult)
    A, C = sm[:, 6], sm[:, 7]
    e.tensor_scalar(A, ry, g_col, None, ALU.mult)
    e.tensor_tensor(C, mean, A, ALU.mult)
    e.tensor_scalar(C, C, b_col, -1.0, ALU.subtract, ALU.mult)
    return A, C


def _body(nc, tc, g, cs, shared_es):
    from contextlib import ExitStack

    ctx = ExitStack()
    singles = ctx.enter_context(tc.tile_pool(name="singles", bufs=1))
    xpool = ctx.enter_context(tc.tile_pool(name="xpool", bufs=1))
    tp = ctx.enter_context(tc.tile_pool(name="tp", bufs=2))
    scr = ctx.enter_context(tc.tile_pool(name="scr", bufs=2))
    np_ = ctx.enter_context(tc.tile_pool(name="npool", bufs=2))
    pp = ctx.enter_context(tc.tile_pool(name="pp", bufs=1))
    ps = ctx.enter_context(tc.tile_pool(name="ps", bufs=4, space="PSUM"))

    # ---- embedding weights + per-item inputs first (gate the pipeline) ----
    t_wnt = singles.tile([3, D], F16, tag="wnt")
    nc.sync.dma_start(t_wnt[:], g["wnt"][:])
    t_wdt = singles.tile([2, D], F32, tag="wdt")
    nc.sync.dma_start(t_wdt[:], g["wdt"][:])
    t_wint = singles.tile([D, D], F32, tag="wint")
    nc.sync.dma_start(t_wint[:], g["wint"][:])
    t_woutt = singles.tile([D, D], F32, tag="woutt")
    nc.sync.dma_start(t_woutt[:], g["woutt"][:])
    t_sb = singles.tile([D, 52], F32, tag="sblob")
    nc.sync.dma_start(t_sb[:], g["sblob"][:])
    t_b4 = t_sb[:, 0:4]
    t_bw1 = t_sb[:, 4:28].rearrange("p (l f) -> p l f", l=L)
    t_g1, t_b1 = t_sb[:, 28:34], t_sb[:, 34:40]
    t_g2, t_b2 = t_sb[:, 40:46], t_sb[:, 46:52]
    t_ff = singles.tile([P, IPC], F32, tag="ffl")
    nc.sync.dma_start(t_ff[:], _bcast_dram(g["flagf"], P, 0, IPC))
    t_const = singles.tile([P, 1], F32, tag="consts")
    nc.gpsimd.memset(t_const[:, 0:1], -KSHIFT)
    KB = t_const[:, 0:1]

    # ---- layer-0 weights, then es8, then remaining layers ----
    t_wb = []

    def load_layer_weights(l):
        w = singles.tile([D, WCOLS], F16, tag=f"wb_{l}", name=f"wb_{l}")
        nc.sync.dma_start(w[:], g["wblob"][l])
        t_wb.append(w)

    load_layer_weights(0)
    es_tiles = {}
    if shared_es:
        for i in range(IPC):
            es = singles.tile([TCS, TC, S], F8E4, tag=f"es{i}", name=f"es{i}")
            nc.sync.dma_start(es[:, 0:4], g["es8"][0, i, :, 0:4])
            nc.sync.dma_start(es[:, 4:8], g["es8"][0, i, :, 4:8])
            es_tiles[i] = es
    for l in range(1, L):
        load_layer_weights(l)

    def wq(l):
        return t_wb[l][:, 0:D]

    def wkv(l):
        return t_wb[l][:, D : 3 * D]

    def w1(l, fc):
        return t_wb[l][:, 3 * D + fc * P : 3 * D + (fc + 1) * P]

    def w2(l, fc):
        return t_wb[l][:, 3 * D + F + fc * P : 3 * D + F + (fc + 1) * P]

    BD, BN_, BIN, BOUT = (t_b4[:, i : i + 1] for i in range(4))
    emb_mean = singles.tile([D, IPC], F32, tag="embm")

    # ---- embedding ----
    x16s = []
    for i in range(IPC):
        x32 = xpool.tile([D, S], F32, tag=f"x32_{i}", name=f"x32_{i}")
        t_node = scr.tile([P, 1024], F16, tag="node16")
        nc.sync.dma_start(t_node[:3, 0:N], g["node_t"][i])
        t_dep = tp.tile([2, 1], F32, tag="dep")
        nc.sync.dma_start(t_dep[:], g["depot"][i, :, None])
        pe = ps.tile([P, 1024], F32, tag="ps")
        nc.tensor.matmul(pe[:, 0:500], t_wnt[:], t_node[:3, 0:500], start=True, stop=True)
        nc.tensor.matmul(pe[:, 512:1011], t_wnt[:], t_node[:3, 500:999], start=True, stop=True)
        nc.scalar.activation(x32[:, 1:501], pe[:, 0:500], AF.Identity, bias=BN_, scale=1.0)
        nc.scalar.activation(x32[:, 501:1000], pe[:, 512:1011], AF.Identity, bias=BN_, scale=1.0)
        pd = ps.tile([P, 1024], F32, tag="ps")
        nc.tensor.matmul(pd[:, 0:1], t_wdt[:], t_dep[:], start=True, stop=True)
        nc.scalar.activation(x32[:, 0:1], pd[:, 0:1], AF.Identity, bias=BD, scale=1.0)
        pw = ps.tile([P, 1024], F32, tag="ps")
        nc.tensor.matmul(pw[:, 0:1], t_wint[:], x32[:, 1:2], start=True, stop=True)
        nc.scalar.activation(x32[:, 1:2], pw[:, 0:1], AF.Identity, bias=BIN, scale=1.0)
        # flag row fix: u = f*x0 + (1-f)*x999 ; w = Wout@u + bout ;
        # x0 += f*(w-u) ; x999 += (1-f)*(w-u)
        fcol = t_ff[:, i : i + 1]
        sm = np_.tile([D, 8], F32, tag="flagtmp")
        d1, u, t2, w_sb, d0 = (sm[:, j : j + 1] for j in range(5))
        nc.vector.tensor_tensor(d1, x32[:, 0:1], x32[:, 999:1000], ALU.subtract)
        nc.vector.tensor_scalar(d1, d1, fcol, None, ALU.mult)
        nc.vector.tensor_tensor(u, x32[:, 999:1000], d1, ALU.add)
        pf = ps.tile([P, 1024], F32, tag="ps")
        nc.tensor.matmul(pf[:, 0:1], t_woutt[:], u, start=True, stop=True)
        nc.scalar.activation(w_sb, pf[:, 0:1], AF.Identity, bias=BOUT, scale=1.0)
        nc.vector.tensor_tensor(t2, w_sb, u, ALU.subtract)
        nc.vector.tensor_scalar(d0, t2, fcol, None, ALU.mult)
        nc.vector.tensor_tensor(x32[:, 0:1], x32[:, 0:1], d0, ALU.add)
        nc.vector.tensor_tensor(x32[:, 999:1000], x32[:, 999:1000], t2, ALU.add)
        nc.vector.tensor_tensor(x32[:, 999:1000], x32[:, 999:1000], d0, ALU.subtract)
        x16 = xpool.tile([D, S], F16, tag=f"x16_{i}", name=f"x16_{i}")
        nc.vector.tensor_copy(x16[:], x32[:])
        nc.vector.tensor_reduce(emb_mean[:, i : i + 1], x32[:], axis=mybir.AxisListType.X, op=ALU.add)
        x16s.append(x16)
    nc.vector.tensor_scalar(emb_mean[:], emb_mean[:], 1.0 / S, None, ALU.mult)

    # ---- encoder layers ----
    for l in range(L):
        wgts, afts, ys, h16s, y2s = {}, {}, {}, {}, {}
        sts = {}
        tqs, eks, ekvs = {}, {}, {}

        def a1(i):
            """q + k|v matmuls, tanh, ek, ekv for item i."""
            g0 = (i // GRP) * GRP
            if i == g0:
                sts[g0] = (np_.tile([D, 2, GRP], F32, tag=f"st1_{g0}", name=f"st1_{g0}"),
                           np_.tile([D, 2, GRP], F32, tag=f"st2_{g0}", name=f"st2_{g0}"))
            x16 = x16s[i]
            pq = ps.tile([P, 1024], F32, tag="ps")
            nc.tensor.matmul(pq[:, 0:500], wq(l), x16[:, 0:500], start=True, stop=True)
            nc.tensor.matmul(pq[:, 512:1012], wq(l), x16[:, 500:1000], start=True, stop=True)
            pkv_a = ps.tile([P, 1024], F32, tag="ps")
            pkv_b = ps.tile([P, 1024], F32, tag="ps")
            for c in range(TC):
                pkv = pkv_a if c < 4 else pkv_b
                cc = c % 4
                nc.tensor.matmul(pkv[:TCS, cc * 256 : (cc + 1) * 256],
                                 x16[:, c * TCS : (c + 1) * TCS], wkv(l),
                                 start=True, stop=True)
            tq = tp.tile([P, S], F16, tag="tq")
            nc.scalar.activation(_v2(tq), _nv(pq), AF.Tanh, bias=0.0, scale=0.5)
            tqs[i] = tq
            ek = tp.tile([P, TC * P], F8E5, tag="ek")
            ekv = tp.tile([P, TC * P], F8E5, tag="ekv")
            for h, pkv in ((0, pkv_a), (1, pkv_b)):
                pv4 = pkv[:TCS].rearrange("p (c two d) -> p c two d", two=2, d=P)
                eko = ek[:TCS, h * 512 : (h + 1) * 512].rearrange("p (c d) -> p c d", d=P)
                ekvo = ekv[:TCS, h * 512 : (h + 1) * 512].rearrange("p (c d) -> p c d", d=P)
                nc.scalar.activation(eko, pv4[:, :, 0], AF.Exp, bias=KB[:TCS], scale=1.0)
                nc.vector.tensor_tensor(ekvo, eko, pv4[:, :, 1], ALU.mult)
            eks[i], ekvs[i] = ek, ekv

        def a2(i):
            """den/num DoubleRow matmuls, reciprocal, wgt, aft for item i."""
            g0 = (i // GRP) * GRP
            ek, ekv = eks[i], ekvs[i]
            if shared_es:
                es = es_tiles[i]
            else:
                es = tp.tile([TCS, TC, S], F8E4, tag="es_dyn")
                nc.sync.dma_start(es[:], g["es8"][l, i])
            pden = ps.tile([P, 1024], F32, tag="ps")
            for sc in range(SC):
                off = sc * 512
                ssl = slice(sc * SCS, (sc + 1) * SCS)
                for c in range(PC):
                    nc.tensor.matmul(pden[:, off : off + SCS],
                                     ek[:TCS, c * 2 * P : (c + 1) * 2 * P].rearrange("p (j d) -> p j d", j=2),
                                     es[:, 2 * c : 2 * c + 2, ssl],
                                     start=(c == 0), stop=(c == PC - 1), perf_mode=DR)
            rden = scr.tile([P, 1024], F32, tag="rden")
            nc.vector.reciprocal_approx_fast(out=_nv(rden), in_=_nv(pden))
            pnum = ps.tile([P, 1024], F32, tag="ps")
            for sc in range(SC):
                off = sc * 512
                ssl = slice(sc * SCS, (sc + 1) * SCS)
                for c in range(PC):
                    nc.tensor.matmul(pnum[:, off : off + SCS],
                                     ekv[:TCS, c * 2 * P : (c + 1) * 2 * P].rearrange("p (j d) -> p j d", j=2),
                                     es[:, 2 * c : 2 * c + 2, ssl],
                                     start=(c == 0), stop=(c == PC - 1), perf_mode=DR)
            wgt = pp.tile([P, S], F16, tag=f"wgt_{i}", name=f"wgt_{i}")
            nc.vector.tensor_tensor(_v2(wgt), _nv(pnum), _nv(rden), ALU.mult)
            wgts[i] = wgt
            st1 = sts[g0][0]
            aft = pp.tile([P, S], F16, tag=f"aft_{i}", name=f"aft_{i}")
            nc.vector.affine_mul_reduce(out=aft[:], accum_out=st1[:, 0, (i - g0) : (i - g0) + 1],
                                        in0=tqs[i][:], in1=wgt[:], scale=0.5, bias=0.5)
            afts[i] = aft

        def phase_b(g0):
            """norm-1 for group g0: y, sumsq, smalls, h16."""
            st1 = sts[g0][0]
            for i in range(g0, g0 + GRP):
                j = i - g0
                y = xpool.tile([P, S], F32, tag=f"y_{i}", name=f"y_{i}")
                nc.vector.tensor_tensor(y[:], x16s[i][:], afts[i][:], ALU.add)
                ys[i] = y
                sq = scr.tile([P, 1024], F32, tag="sqscr")
                nc.scalar.activation(sq[:, 0:S], y[:], AF.Square, accum_out=st1[:, 1, j : j + 1])
            mb = t_b2[:, l - 1 : l] if l > 0 else None
            mbc = emb_mean[:, g0 : g0 + GRP] if l == 0 else None
            A1, C1 = _norm_smalls(nc, np_, st1[:, 0], st1[:, 1],
                                  t_g1[:, l : l + 1], t_b1[:, l : l + 1], f"n1_{g0}",
                                  mean_bias=mb, mean_bias_cols=mbc)
            for i in range(g0, g0 + GRP):
                j = i - g0
                h16 = pp.tile([P, S], F16, tag=f"h16_{i}", name=f"h16_{i}")
                nc.gpsimd.tensor_scalar(h16[:], ys[i][:], A1[:, j : j + 1], C1[:, j : j + 1],
                                        ALU.mult, ALU.add)
                h16s[i] = h16

        def phase_c(i):
            """FFN + y2 + sumsq for item i (fc-pipelined emission)."""
            g0 = (i // GRP) * GRP
            j = i - g0
            st2 = sts[g0][1]
            h16 = h16s[i]
            pf2 = ps.tile([P, 1024], F32, tag="ps")
            for fc in range(FC):
                pf1 = ps.tile([P, 1024], F32, tag="ps")
                nc.tensor.matmul(pf1[:, 0:500], w1(l, fc), h16[:, 0:500], start=True, stop=True)
                nc.tensor.matmul(pf1[:, 512:1012], w1(l, fc), h16[:, 500:1000], start=True, stop=True)
                r16 = tp.tile([P, S], F16, tag="r16")
                bcol = t_bw1[:, l, fc : fc + 1]
                if fc % 2 == 0:
                    nc.scalar.activation(_v2(r16), _nv(pf1), AF.Relu, bias=bcol, scale=1.0)
                else:
                    nc.vector.tensor_scalar(_v2(r16), _nv(pf1), bcol, 0.0, ALU.add, ALU.max)
                for sc in range(SC):
                    nc.tensor.matmul(pf2[:, sc * 512 : sc * 512 + SCS],
                                     w2(l, fc), r16[:, sc * SCS : (sc + 1) * SCS],
                                     start=(fc == 0), stop=(fc == FC - 1))
            # y2 = h + ff  (bW2 cancels in next norm)
            y2 = xpool.tile([P, S], F32, tag=f"y2_{i}", name=f"y2_{i}")
            nc.vector.scalar_tensor_tensor(_v2(y2), _v2(h16), 0.0, _nv(pf2),
                                           ALU.add, ALU.add,
                                           accum_out=st2[:, 0, j : j + 1])
            y2s[i] = y2
            sq = scr.tile([P, 1024], F32, tag="sqscr")
            nc.scalar.activation(sq[:, 0:S], y2[:], AF.Square, accum_out=st2[:, 1, j : j + 1])

        def phase_d(g0):
            """norm-2 for group g0: smalls + next-layer x16 (or output)."""
            st2 = sts[g0][1]
            A2, C2 = _norm_smalls(nc, np_, st2[:, 0], st2[:, 1],
                                  t_g2[:, l : l + 1], t_b2[:, l : l + 1], f"n2_{g0}")
            for i in range(g0, g0 + GRP):
                j = i - g0
                if l < L - 1:
                    nx16 = xpool.tile([D, S], F16, tag=f"x16_{i}{'b' if l % 2 == 0 else ''}",
                                      name=f"nx16_{i}")
                    nc.gpsimd.tensor_scalar(nx16[:], y2s[i][:], A2[:, j : j + 1], C2[:, j : j + 1],
                                            ALU.mult, ALU.add)
                    x16s[i] = nx16
                else:
                    xout = xpool.tile([D, S], F32, tag=f"x32_{i}", name=f"xout_{i}")
                    nc.gpsimd.tensor_scalar(xout[:], y2s[i][:], A2[:, j : j + 1], C2[:, j : j + 1],
                                            ALU.mult, ALU.add)
                    nc.sync.dma_start(g["out32"][i], xout[:])

        # v3 emission order: full attention per item, then norms, then FFN
        for i in range(IPC):
            a1(i)
            a2(i)
        phase_b(0)
        phase_b(2)
        phase_c(0)
        phase_c(1)
        phase_d(0)
        phase_c(2)
        phase_c(3)
        phase_d(2)

    ctx.close()


# ------------------------------------------------------------------
# host wrapper
# ------------------------------------------------------------------
_cache = {}


def _get_nc(cs_key):
    if cs_key not in _cache:
        _cache[cs_key] = build_cvrp(list(cs_key))
    return _cache[cs_key]


def prep_inputs(depot_xy, node_xy_demand, dist, log_scale, flag,
                Wd, bd, Wn, bn, Win, bin_, Wout, bout,
                Wq, Wk, Wv, alpha, g1, b1, W1, bW1, W2, bW2, g2, b2):
    import ml_dtypes

    flag = np.asarray(flag)
    cs = tuple(float(np.asarray(log_scale)[0]) * float(a) for a in np.asarray(alpha))
    shared_es = all(abs(c - cs[0]) < 1e-30 for c in cs)

    # es8[b, p, cc, s] = exp(-c_l * dist[b, s, cc*125+p]), fp8-e4m3,
    # cc = consecutive 125-row t-chunks (DoubleRow pairs are (2c, 2c+1)).
    dist_t = np.asarray(dist).transpose(0, 2, 1).reshape(B, TC, TCS, S)
    layers = [cs[0]] if shared_es else list(cs)
    es8 = np.empty((len(layers), B, TCS, TC, S), dtype=ml_dtypes.float8_e4m3)
    for li, c in enumerate(layers):
        es8[li] = np.exp(-c * dist_t).transpose(0, 2, 1, 3).astype(ml_dtypes.float8_e4m3)

    node_t = np.ascontiguousarray(np.asarray(node_xy_demand).transpose(0, 2, 1)).astype(np.float16)
    depot = np.asarray(depot_xy).reshape(B, 2).astype(np.float32)
    flagf = flag.astype(np.float32)

    f16 = lambda a: np.ascontiguousarray(np.asarray(a)).astype(np.float16)
    f32 = lambda a: np.ascontiguousarray(np.asarray(a)).astype(np.float32)
    # per-layer weight blob: [L, D, 1408] = WqT | WkT | WvT | W1T | W2blob
    w2b = np.asarray(W2).transpose(0, 2, 1).reshape(L, FC, P, D).transpose(0, 2, 1, 3).reshape(L, P, FC * D)
    wblob = np.concatenate([
        np.asarray(Wq).transpose(0, 2, 1),
        np.asarray(Wk).transpose(0, 2, 1),
        np.asarray(Wv).transpose(0, 2, 1),
        np.asarray(W1).transpose(0, 2, 1),
        w2b,
    ], axis=2).astype(np.float16)
    sblob = np.concatenate([
        np.stack([np.asarray(bd), np.asarray(bn), np.asarray(bin_), np.asarray(bout)], axis=1),
        np.asarray(bW1).reshape(L, FC, P).transpose(2, 0, 1).reshape(P, L * FC),
        np.asarray(g1).T, np.asarray(b1).T, np.asarray(g2).T, np.asarray(b2).T,
    ], axis=1).astype(np.float32)
    shared = {
        "wblob": np.ascontiguousarray(wblob),
        "wnt": f16(np.asarray(Wn).T),
        "wdt": f32(np.asarray(Wd).T),
        "wint": f32(np.asarray(Win).T),
        "woutt": f32(np.asarray(Wout).T),
        "sblob": np.ascontiguousarray(sblob),
    }
    in_maps = []
    for c in range(NCORES):
        sl = slice(c * IPC, (c + 1) * IPC)
        m = dict(shared)
        m["es8"] = np.ascontiguousarray(es8[:, sl])
        m["node_t"] = node_t[sl]
        m["depot"] = depot[sl]
        m["flagf"] = flagf[sl]
        in_maps.append(m)
    return cs, in_maps


TRACE = False
LAST_RESULT = None


def kernel(**inputs):
    global LAST_RESULT
    cs, in_maps = prep_inputs(**inputs)
    nc = _get_nc(cs)
    res = run_bass_kernel_spmd(nc, in_maps, list(range(NCORES)), trace=TRACE)
    LAST_RESULT = res
    out = np.concatenate([r["out32"] for r in res.results], axis=0)  # [B, D, S]
    return np.ascontiguousarray(out.transpose(0, 2, 1)).astype(np.float32)
